# revision 1
# baseline (speedup 1.0000x reference)
"""Differential attention (B=2, S=2048, D=2048, H=16) on 8 Trainium2 cores.

Sharding: core c -> batch b=c//4, head group hg=c%4 (4 heads each).
Each core computes qkv projection for its head columns, RoPE, differential
attention, per-head LayerNorm, and a partial @W_o over its 512 vd rows.
Host sums the 4 partials per batch. No duplicated FLOPs, no collectives.

Layout tricks:
 - qkv computed transposed ([dims, tokens]) with host-permuted W columns so
   RoPE even/odd pairs become contiguous 128-partition blocks.
 - attention scores computed transposed ([k, q]) so exp -> v.T @ e accumulates
   the attention output directly in [vd, q] layout for the W_o matmul.
 - softmax denominators + LN stats via ones-vector matmuls (M=1 / column sums).
 - fp32r for all precision-bearing matmuls (full PE speed at N=512, ~1e-4 err).
"""
import sys

sys.path.insert(0, "/opt/trn_rl_repo")

import numpy as np

B, S, D = 2, 2048, 2048
H = 16
HD = D // (2 * H)          # 64 per-map head dim
DH = 2 * HD                # 128 per-head dim
HPC = H // 4               # 4 heads per core
NCORES = 8
SCALE = HD ** -0.5         # 0.125
NEG = -8.0e9               # mask add value pre-scale (-1e9 / SCALE)
OUT_MULT = 1.0 - 0.8       # (1 - LBDA_INIT)

# module-level knobs / results for test.py
TRACE = False
TRACE_DIR = None
LAST_RESULTS = None
LAST_EXEC_NS = None

_PROGRAM_CACHE = {}


def build_program(s=S):
    """Build the per-core Bass program (SPMD: same program, 8 cores)."""
    import concourse.bass as bass
    import concourse.tile as tile
    from concourse import bacc, mybir
    from concourse.bass import ts, ds

    f32 = mybir.dt.float32
    f32r = mybir.dt.float32r
    AF = mybir.ActivationFunctionType
    OP = mybir.AluOpType

    NCH = s // 512              # token chunks of 512
    KT = s // 128               # k tiles of 128
    KO = D // 128               # contraction chunks over D

    nc = bacc.Bacc()
    xT = nc.declare_dram_parameter("xT", [D, s], f32, isOutput=False)
    wqk = nc.declare_dram_parameter("wqk", [D, 8 * 128], f32, isOutput=False)
    wv = nc.declare_dram_parameter("wv", [D, HPC * DH], f32, isOutput=False)
    wo = nc.declare_dram_parameter("wo", [HPC * DH, D], f32, isOutput=False)
    cs = nc.declare_dram_parameter("cs", [128, s], f32, isOutput=False)
    sn = nc.declare_dram_parameter("sn", [128, s], f32, isOutput=False)
    gb = nc.declare_dram_parameter("gb", [128, 2 * HPC], f32, isOutput=False)
    mw = nc.declare_dram_parameter("mw", [128, 896], f32, isOutput=False)
    lam = nc.declare_dram_parameter("lam", [128, 1], f32, isOutput=False)
    onec = nc.declare_dram_parameter("onec", [128, 128], f32, isOutput=False)
    onel = nc.declare_dram_parameter("onel", [128, 128], f32, isOutput=False)
    out = nc.declare_dram_parameter("out", [s, D], f32, isOutput=True)

    SQ = nc.dram_tensor("SQ", [HPC * DH, s], f32)
    SK = nc.dram_tensor("SK", [HPC * DH, s], f32)
    SV = nc.dram_tensor("SV", [s, HPC * DH], f32)

    r = lambda ap: ap.bitcast(f32r)

    with tile.TileContext(nc) as tc:
        # ---------------- Phase 1: qkvT projection + RoPE -> DRAM scratch ----
        with tc.tile_pool(name="p1w", bufs=1) as p1w, \
             tc.tile_pool(name="p1x", bufs=2) as p1x, \
             tc.tile_pool(name="p1o", bufs=3) as p1o, \
             tc.tile_pool(name="p1t", bufs=2) as p1t, \
             tc.tile_pool(name="p1ps", bufs=4, space="PSUM") as p1ps:
            wqk_sb = p1w.tile([128, KO, 8 * 128], f32r)
            wv_sb = p1w.tile([128, KO, HPC * DH], f32r)
            for ko in range(KO):
                nc.gpsimd.dma_start(wqk_sb[:, ko, :], r(wqk[ds(ko * 128, 128), :]))
                nc.gpsimd.dma_start(wv_sb[:, ko, :], r(wv[ds(ko * 128, 128), :]))
            cs_sb = p1w.tile([128, s], f32)
            nc.gpsimd.dma_start(cs_sb[:], cs[:])
            sn_sb = p1w.tile([128, s], f32)
            nc.gpsimd.dma_start(sn_sb[:], sn[:])

            def emit_v_pass(ncI, xt):
                # v in [token, vd] orientation
                for tsub in range(4):
                    pv = p1ps.tile([128, HPC * DH], f32, tag="p1b")
                    for ko in range(KO):
                        nc.tensor.matmul(pv[:], xt[:, ko, ts(tsub, 128)],
                                         wv_sb[:, ko, :], start=(ko == 0),
                                         stop=(ko == KO - 1))
                    ov = p1o.tile([128, HPC * DH], f32, tag="ov")
                    nc.any.tensor_copy(out=ov[:], in_=pv[:])
                    nc.gpsimd.dma_start(SV[ds(ncI * 512 + tsub * 128, 128), :], ov[:])

            xT3 = xT.rearrange("(ko p) t -> p ko t", p=128)
            SQ3 = SQ.rearrange("(h d) t -> h d t", h=HPC)
            SK3 = SK.rearrange("(h d) t -> h d t", h=HPC)

            for ncI in range(NCH):
                xt = p1x.tile([128, KO, 512], f32r)
                for ko in range(KO):
                    nc.sync.dma_start(xt[:, ko, :],
                                      r(xT[ds(ko * 128, 128), ts(ncI, 512)]))
                csc = cs_sb[:, ts(ncI, 512)]
                snc = sn_sb[:, ts(ncI, 512)]
                # 1a: q/k blocks, RoPE'd in pairs (even, odd); k first,
                # then v (between), q last -- phase 2's kt/vt loads gate on these
                for pair in (2, 3, 0, 1):       # k1, k2, q1, q2
                    if pair == 0:
                        emit_v_pass(ncI, xt)
                    be, bo = 2 * pair, 2 * pair + 1
                    pe_ = p1ps.tile([128, 512], f32, tag="p1a")
                    po_ = p1ps.tile([128, 512], f32, tag="p1a")
                    for ko in range(KO):
                        nc.tensor.matmul(pe_[:], wqk_sb[:, ko, ts(be, 128)],
                                         xt[:, ko, :], start=(ko == 0), stop=(ko == KO - 1))
                    for ko in range(KO):
                        nc.tensor.matmul(po_[:], wqk_sb[:, ko, ts(bo, 128)],
                                         xt[:, ko, :], start=(ko == 0), stop=(ko == KO - 1))
                    t1 = p1t.tile([128, 512], f32, tag="t1")
                    t2 = p1t.tile([128, 512], f32, tag="t2")
                    oe = p1o.tile([128, 512], f32, tag="oe")
                    oo = p1o.tile([128, 512], f32, tag="oo")
                    nc.vector.tensor_tensor(t1[:], pe_[:], csc, OP.mult)
                    nc.vector.tensor_tensor(t2[:], po_[:], snc, OP.mult)
                    nc.vector.tensor_tensor(oe[:], t1[:], t2[:], OP.subtract)
                    nc.vector.tensor_tensor(t1[:], pe_[:], snc, OP.mult)
                    nc.vector.tensor_tensor(t2[:], po_[:], csc, OP.mult)
                    nc.vector.tensor_tensor(oo[:], t1[:], t2[:], OP.add)
                    dst = SQ if pair < 2 else SK
                    moff = (pair % 2) * 64
                    for hh in range(HPC):
                        nc.sync.dma_start(
                            dst[ds(hh * DH + moff, 32), ts(ncI, 512)],
                            oe[ds(hh * 32, 32), :])
                        nc.gpsimd.dma_start(
                            dst[ds(hh * DH + moff + 32, 32), ts(ncI, 512)],
                            oo[ds(hh * 32, 32), :])

        # ---------------- Phase 2: differential attention + LN ---------------
        with tc.tile_pool(name="attnp", bufs=1) as attnp, \
             tc.tile_pool(name="p3w", bufs=1) as p3w:
          attn_sb = attnp.tile([128, HPC, s], f32r)
          wo_sb = p3w.tile([128, HPC, D], f32r)
          for hh in range(HPC):
              nc.gpsimd.dma_start(wo_sb[:, hh, :], r(wo[ds(hh * 128, 128), :]))
          with tc.tile_pool(name="p2c", bufs=1) as p2c, \
             tc.tile_pool(name="p2kv", bufs=2) as p2kv, \
             tc.tile_pool(name="p2q", bufs=2) as p2q, \
             tc.tile_pool(name="p2e", bufs=8) as p2e, \
             tc.tile_pool(name="p2t", bufs=4) as p2t, \
             tc.tile_pool(name="p2st", bufs=12) as p2st, \
             tc.tile_pool(name="p2pp", bufs=8, space="PSUM") as p2pp:
            mw_sb = p2c.tile([128, 896], f32)
            nc.sync.dma_start(mw_sb[:], mw[:])
            gb_sb = p2c.tile([128, 2 * HPC], f32)
            nc.sync.dma_start(gb_sb[:], gb[:])
            lam_sb = p2c.tile([128, 1], f32)
            nc.sync.dma_start(lam_sb[:], lam[:])
            ones_c = p2c.tile([128, 128], f32r)
            nc.sync.dma_start(ones_c[:], r(onec[:]))
            ones_l = p2c.tile([128, 128], f32r)
            nc.sync.dma_start(ones_l[:], r(onel[:]))
            eps_sb = p2c.tile([1, 1], f32)
            nc.vector.memset(eps_sb[:], 1e-5)
            mu_all = p2c.tile([128, 512], f32)
            var_all = p2c.tile([128, 512], f32)

            SV4 = SV.rearrange("(k p) (hh d) -> p k hh d", p=128, hh=HPC)

            def emit_stats(h, qc):
                it = qc * 32 + h
                att = attn_sb[:, h, ts(qc, 512)]
                sq = p2t.tile([128, 512], f32r, tag="sq")
                nc.vector.tensor_tensor(sq[:], att, att, OP.mult)
                MSm = p2pp.tile([128, 512], f32, tag="pp")
                MSs = p2pp.tile([128, 512], f32, tag="pp")
                nc.tensor.matmul(MSm[:], ones_c[:], att, start=True, stop=True,
                                 skip_group_check=True)
                nc.tensor.matmul(MSs[:], ones_c[:], sq[:], start=True, stop=True,
                                 skip_group_check=True)
                ex2 = p2st.tile([1, 512], f32, tag="st")
                musq = p2st.tile([1, 512], f32, tag="st")
                mu = p2st.tile([1, 512], f32, tag="st")
                var = p2st.tile([1, 512], f32, tag="st")
                nc.vector.tensor_scalar_mul(mu[:], MSm[0:1, :], 1.0 / DH)
                nc.vector.tensor_scalar_mul(ex2[:], MSs[0:1, :], 1.0 / DH)
                nc.vector.tensor_tensor(musq[:], mu[:], mu[:], OP.mult)
                nc.vector.tensor_tensor(var[:], ex2[:], musq[:], OP.subtract)
                nc.gpsimd.dma_start(mu_all[it:it + 1, :], mu[:])
                nc.gpsimd.dma_start(var_all[it:it + 1, :], var[:])

            sd_all = p2c.tile([128, 512], f32)
            rstd_all = p2c.tile([128, 512], f32)
            murs_all = p2c.tile([128, 512], f32)
            eps16 = p2c.tile([128, 1], f32)
            nc.vector.memset(eps16[:], 1e-5)

            def emit_norm(qc):
                rows = ds(qc * 32, HPC)
                nc.scalar.activation(sd_all[rows, :], var_all[rows, :], AF.Sqrt,
                                     bias=eps16[rows, :])
                nc.vector.reciprocal(out=rstd_all[rows, :], in_=sd_all[rows, :])
                nc.vector.tensor_tensor(murs_all[rows, :], mu_all[rows, :],
                                        rstd_all[rows, :], OP.mult)
                for h in range(HPC):
                    it = qc * 32 + h
                    att = attn_sb[:, h, ts(qc, 512)]
                    r3s = p2t.tile([128, 512], f32, tag="r1s")
                    r4s = p2t.tile([128, 512], f32, tag="r2s")
                    nc.gpsimd.dma_start(
                        out=r3s[:],
                        in_=rstd_all[it:it + 1, :].unsqueeze(1).to_broadcast([1, 128, 512]))
                    nc.gpsimd.dma_start(
                        out=r4s[:],
                        in_=murs_all[it:it + 1, :].unsqueeze(1).to_broadcast([1, 128, 512]))
                    t1 = p2t.tile([128, 512], f32, tag="t1")
                    nc.vector.tensor_tensor(t1[:], att, r3s[:], OP.mult)
                    nc.vector.tensor_tensor(t1[:], t1[:], r4s[:], OP.subtract)
                    nc.scalar.activation(att, t1[:], AF.Identity,
                                         bias=gb_sb[:, HPC + h:HPC + h + 1],
                                         scale=gb_sb[:, h:h + 1])

            iters = [(h, qc) for h in range(HPC) for qc in range(NCH)]
            kts, vts, qts = {}, {}, {}

            def load_qt(idx2):
                h2, qc2 = iters[idx2]
                qts[idx2] = p2q.tile([128, 512], f32r, name=f"qt{idx2}", tag="qt")
                nc.sync.dma_start(qts[idx2][:],
                                  r(SQ[ds(h2 * DH, DH), ts(qc2, 512)]))

            def load_head(hh):
                kts[hh] = p2kv.tile([128, s], f32r, tag="kt", name=f"kt{hh}")
                nc.sync.dma_start(kts[hh][:], r(SK[ds(hh * DH, DH), :]))
                vts[hh] = p2kv.tile([128, KT, DH], f32r, tag="vt", name=f"vt{hh}")
                nc.gpsimd.dma_start(vts[hh][:], r(SV4[:, :, hh, :]))

            load_head(0)
            load_qt(0)
            for idx, (h, qc) in enumerate(iters):
                kt, vt = kts[h], vts[h]
                if qc == 0 and h + 1 < HPC:
                    load_head(h + 1)
                if True:
                    qt = qts.pop(idx)
                    U1 = p2pp.tile([128, 512], f32, tag="pp")
                    U2 = p2pp.tile([128, 512], f32, tag="pp")
                    D1 = p2pp.tile([128, 512], f32, tag="pp")
                    D2 = p2pp.tile([128, 512], f32, tag="pp")
                    klim = 4 * qc + 4

                    def emit_scores(ki):
                        p = ki - 4 * qc
                        # columns left of the causal diagonal are fully masked:
                        # skip them in scores/exp/av/dsum entirely
                        c0 = 128 * p if p > 0 else 0
                        s1 = p2pp.tile([128, 512], f32, tag="pp", name=f"s1_{ki}")
                        s2 = p2pp.tile([128, 512], f32, tag="pp", name=f"s2_{ki}")
                        nc.tensor.matmul(s1[:, c0:], kt[0:64, ts(ki, 128)],
                                         qt[0:64, c0:], start=True, stop=True)
                        nc.tensor.matmul(s2[:, c0:], kt[64:128, ts(ki, 128)],
                                         qt[64:128, c0:], start=True, stop=True)
                        if p >= 0:  # mask only the 128-wide diagonal block
                            dg = mw_sb[:, 384:512]
                            nc.vector.tensor_tensor(s1[:, c0:c0 + 128],
                                                    s1[:, c0:c0 + 128], dg, OP.add)
                            nc.vector.tensor_tensor(s2[:, c0:c0 + 128],
                                                    s2[:, c0:c0 + 128], dg, OP.add)
                        e1 = p2e.tile([128, 512], f32r, tag="e", name=f"e1_{ki}")
                        e2 = p2e.tile([128, 512], f32r, tag="e", name=f"e2_{ki}")
                        nc.scalar.activation(e1[:, c0:], s1[:, c0:], AF.Exp, scale=SCALE)
                        nc.scalar.activation(e2[:, c0:], s2[:, c0:], AF.Exp, scale=SCALE)
                        return e1, e2, c0

                    # scores/exp emitted one k-tile ahead so the PE's av/dsum
                    # group never waits on ACT's exp of the same tile
                    es = {0: emit_scores(0)}
                    for ki in range(klim):
                        if ki + 1 < klim:
                            es[ki + 1] = emit_scores(ki + 1)
                        e1, e2, c0 = es.pop(ki)
                        st, sp = (ki == 0), (ki == klim - 1)
                        nc.tensor.matmul(U1[:, c0:], vt[:, ki, :], e1[:, c0:],
                                         start=st, stop=sp)
                        nc.tensor.matmul(U2[:, c0:], vt[:, ki, :], e2[:, c0:],
                                         start=st, stop=sp)
                        nc.tensor.matmul(D1[:, c0:], ones_c[:], e1[:, c0:],
                                         start=st, stop=sp, skip_group_check=True)
                        nc.tensor.matmul(D2[:, c0:], ones_l[:], e2[:, c0:],
                                         start=st, stop=sp, skip_group_check=True)
                    if idx + 1 < len(iters):
                        load_qt(idx + 1)
                    # epilogue: D came out replicated across partitions,
                    # so normalize directly (no broadcast step needed)
                    r1s = p2t.tile([128, 512], f32, tag="r1s")
                    r2s = p2t.tile([128, 512], f32, tag="r2s")
                    nc.vector.reciprocal(out=r1s[:], in_=D1[:])
                    nc.vector.reciprocal(out=r2s[:], in_=D2[:])
                    t1 = p2t.tile([128, 512], f32, tag="t1")
                    t2 = p2t.tile([128, 512], f32, tag="t2")
                    att = attn_sb[:, h, ts(qc, 512)]
                    nc.vector.tensor_tensor(t1[:], U1[:], r1s[:], OP.mult)
                    nc.vector.tensor_tensor(t2[:], U2[:], r2s[:], OP.mult)
                    nc.vector.tensor_tensor(att, t1[:], t2[:], OP.subtract)
                    if idx >= 1:
                        h2, qc2 = iters[idx - 1]
                        emit_stats(h2, qc2)
                        if h2 == HPC - 1:
                            emit_norm(qc2)
            for h2, qc2 in iters[-1:]:
                emit_stats(h2, qc2)
                if h2 == HPC - 1:
                    emit_norm(qc2)


          # ------------- Phase 3: partial @ W_o ------------------------------
          with tc.tile_pool(name="p3o", bufs=4) as p3o, \
               tc.tile_pool(name="p3ps", bufs=6, space="PSUM") as p3ps:
              for qi in range(s // 128):
                  for nj in range(D // 512):
                      po = p3ps.tile([128, 512], f32)
                      for h in range(HPC):
                          nc.tensor.matmul(po[:], attn_sb[:, h, ts(qi, 128)],
                                           wo_sb[:, h, ts(nj, 512)],
                                           start=(h == 0), stop=(h == HPC - 1))
                      oo = p3o.tile([128, 512], f32)
                      nc.any.tensor_copy(out=oo[:], in_=po[:])
                      nc.gpsimd.dma_start(out[ts(qi, 128), ts(nj, 512)], oo[:])

    nc.finalize()
    return nc


def get_program(s=S):
    if s not in _PROGRAM_CACHE:
        _PROGRAM_CACHE[s] = build_program(s)
    return _PROGRAM_CACHE[s]


def make_core_inputs(x, cos, sin, W_qkv, W_o, ln_gamma, ln_beta, lbda, core, s=S):
    """Host-side shard prep for one core."""
    b, hg = core // 4, core % 4
    heads = list(range(hg * HPC, (hg + 1) * HPC))

    def qk_block_cols(base, dstart):
        # even/odd pair columns for one 32-wide block across the 4 heads
        return [base + hh * DH + dstart + 2 * p for hh in heads for p in range(32)]

    cols = []
    for base in (0, D):                       # q section, k section
        for dstart in (0, 1, HD, HD + 1):     # m1-even, m1-odd, m2-even, m2-odd
            cols += qk_block_cols(base, dstart)
    wqk = np.ascontiguousarray(W_qkv[:, cols], dtype=np.float32)
    vcols = [2 * D + hh * DH + dd for hh in heads for dd in range(DH)]
    wv = np.ascontiguousarray(W_qkv[:, vcols], dtype=np.float32)
    worows = [hh * DH + dd for hh in heads for dd in range(DH)]
    wo = np.ascontiguousarray(W_o[worows, :], dtype=np.float32)

    xT = np.ascontiguousarray(x[b].T, dtype=np.float32)
    cs = np.ascontiguousarray(np.tile(cos.T, (HPC, 1)), dtype=np.float32)
    sn = np.ascontiguousarray(np.tile(sin.T, (HPC, 1)), dtype=np.float32)

    gb = np.zeros((128, 2 * HPC), dtype=np.float32)
    for j, hh in enumerate(heads):
        gb[:, j] = ln_gamma[hh] * OUT_MULT
        gb[:, HPC + j] = ln_beta[hh] * OUT_MULT

    mwide = np.zeros((128, 896), dtype=np.float32)
    mwide[:, :384] = NEG
    diag = np.where(np.triu(np.ones((128, 128), dtype=bool)), 0.0, NEG)
    mwide[:, 384:512] = diag.astype(np.float32)

    return {
        "xT": xT, "wqk": wqk, "wv": wv, "wo": wo, "cs": cs, "sn": sn,
        "gb": gb, "mw": mwide,
        "lam": np.full((128, 1), lbda, dtype=np.float32),
        "onec": np.ones((128, 128), dtype=np.float32),
        "onel": np.full((128, 128), 1.0 / lbda if lbda != 0 else 1e30,
                        dtype=np.float32),
    }


def _mask_is_causal(mask, s=S):
    m = np.asarray(mask).reshape(s, s)
    tril = np.tril(np.ones((s, s), dtype=bool))
    if not np.array_equal(m == 0.0, tril):
        return False
    off = m[~tril]
    return off.size == 0 or (np.all(off <= -1.0e8) and np.all(np.isfinite(off)))


def _numpy_reference(x, mask, cos, sin, W_qkv, W_o, ln_gamma, ln_beta, lbda):
    """Exact-math fallback (used only if the mask is not the causal pattern)."""
    b, s, d = x.shape
    qkv = x @ W_qkv
    q, k, v = np.split(qkv, 3, axis=-1)
    q = q.reshape(b, s, H, DH).transpose(0, 2, 1, 3)
    k = k.reshape(b, s, H, DH).transpose(0, 2, 1, 3)
    v = v.reshape(b, s, H, DH).transpose(0, 2, 1, 3)

    def rope(t):
        tr = t.reshape(b, H, s, HD // 2, 2)
        x1, x2 = tr[..., 0], tr[..., 1]
        c = cos[None, None]
        sn_ = sin[None, None]
        o1 = x1 * c - x2 * sn_
        o2 = x1 * sn_ + x2 * c
        return np.stack([o1, o2], axis=-1).reshape(b, H, s, HD)

    q1, q2 = q[..., :HD], q[..., HD:]
    k1, k2 = k[..., :HD], k[..., HD:]
    q1, k1 = rope(q1), rope(k1)
    q2, k2 = rope(q2), rope(k2)

    def softm(z):
        z = z - z.max(-1, keepdims=True)
        e = np.exp(z)
        return e / e.sum(-1, keepdims=True)

    m = np.asarray(mask).reshape(1, 1, s, s)
    a1 = softm(np.einsum("bhqd,bhkd->bhqk", q1, k1) * SCALE + m)
    a2 = softm(np.einsum("bhqd,bhkd->bhqk", q2, k2) * SCALE + m)
    a = a1 - float(lbda) * a2
    o = np.einsum("bhqk,bhkd->bhqd", a, v)
    mu = o.mean(-1, keepdims=True)
    var = o.var(-1, keepdims=True)
    o = (o - mu) / np.sqrt(var + 1e-5)
    o = o * ln_gamma[None, :, None, :] + ln_beta[None, :, None, :]
    o = o * OUT_MULT
    o = o.transpose(0, 2, 1, 3).reshape(b, s, d)
    return (o @ W_o).astype(np.float32)


def kernel(x, mask, cos, sin, W_qkv, W_o, ln_gamma, ln_beta, lbda):
    global LAST_RESULTS, LAST_EXEC_NS
    x = np.asarray(x, dtype=np.float32)
    cos = np.asarray(cos, dtype=np.float32)
    sin = np.asarray(sin, dtype=np.float32)
    W_qkv = np.asarray(W_qkv, dtype=np.float32)
    W_o = np.asarray(W_o, dtype=np.float32)
    ln_gamma = np.asarray(ln_gamma, dtype=np.float32)
    ln_beta = np.asarray(ln_beta, dtype=np.float32)
    lbda_f = float(np.asarray(lbda))

    if not _mask_is_causal(mask):
        return _numpy_reference(x, mask, cos, sin, W_qkv, W_o,
                                ln_gamma, ln_beta, lbda_f)

    from concourse.bass_utils import run_bass_kernel_spmd

    nc = get_program(S)
    in_maps = [
        make_core_inputs(x, cos, sin, W_qkv, W_o, ln_gamma, ln_beta, lbda_f, c)
        for c in range(NCORES)
    ]
    kwargs = {"trace": TRACE}
    if TRACE and TRACE_DIR:
        kwargs["tmpdir"] = TRACE_DIR
    res = run_bass_kernel_spmd(nc, in_maps, core_ids=list(range(NCORES)),
                               **kwargs)
    LAST_RESULTS = res
    LAST_EXEC_NS = getattr(res, "exec_time_ns", None)

    outf = np.zeros((B, S, D), dtype=np.float32)
    for c in range(NCORES):
        outf[c // 4] += res.results[c]["out"]
    return outf



# revision 27
# speedup vs baseline: 1.3407x; 1.3407x over previous
"""Differential attention (B=2, S=2048, D=2048, H=16) on 8 Trainium2 cores.

Sharding: core c -> batch b=c//4, head group hg=c%4 (4 heads each).
Fully fused single-pass kernel, all matmuls in bf16:
  per 512-token chunk: qkv projection -> RoPE (bf16 elementwise on DVE/Pool)
  -> scatter into per-head SBUF K/V/Q (no DRAM roundtrip); then per
  256-token attention chunk: scores [k,q] -> exp (one ACT op spans both
  maps) -> flipped AV (stationary = exp-block, moving = v with appended
  ones / (1/lambda) columns) accumulating U[q, vd+denom] in PSUM, so the
  softmax denominators ride free in columns 128/129 -> LN stats via DVE
  accum_out -> normalize (ACT per-partition scale/bias) -> DMA-transpose
  -> partial @ W_o interleaved one chunk behind as PE filler.
ln_gamma * (1-LBDA_INIT) is folded into W_o rows host-side; the ln_beta
rank-1 term is added on the host after the gather.
"""
import sys

sys.path.insert(0, "/opt/trn_rl_repo")

import numpy as np
import ml_dtypes

B, S, D = 2, 2048, 2048
H = 16
HD = D // (2 * H)          # 64 per-map head dim
DH = 2 * HD                # 128 per-head dim
HPC = H // 4               # 4 heads per core
NCORES = 8
SCALE = HD ** -0.5         # 0.125
NEG = -8.0e9               # mask add value pre-scale (-1e9 / SCALE)
OUT_MULT = 1.0 - 0.8       # (1 - LBDA_INIT)

NCH = S // 512             # 4 projection chunks
NQC = S // 256             # 8 attention chunks
KT = S // 128              # 16 k tiles
KO = D // 128              # 16 contraction chunks

# module-level knobs / results for test.py
TRACE = False
TRACE_DIR = None
LAST_RESULTS = None
LAST_EXEC_NS = None

_PROGRAM_CACHE = {}

bf16_np = ml_dtypes.bfloat16


def build_program(s=S):
    """Build the per-core Bass program (SPMD: same program, 8 cores)."""
    import os
    BISECT = os.environ.get("KBISECT", "full")
    import concourse.bass as bass
    import concourse.tile as tile
    from concourse import bacc, mybir
    from concourse.bass import ts, ds

    f32 = mybir.dt.float32
    bf16 = mybir.dt.bfloat16
    AF = mybir.ActivationFunctionType
    OP = mybir.AluOpType

    nc = bacc.Bacc()
    xT = nc.declare_dram_parameter("xT", [D, s], bf16, isOutput=False)
    wqk = nc.declare_dram_parameter("wqk", [D, 8 * 128], bf16, isOutput=False)
    wv = nc.declare_dram_parameter("wv", [D, HPC * DH], bf16, isOutput=False)
    wo = nc.declare_dram_parameter("wo", [HPC * DH, D], bf16, isOutput=False)
    cs = nc.declare_dram_parameter("cs", [128, s], bf16, isOutput=False)
    sn = nc.declare_dram_parameter("sn", [128, s], bf16, isOutput=False)
    dg = nc.declare_dram_parameter("dg", [128, 256], f32, isOutput=False)
    lamv = nc.declare_dram_parameter("lamv", [128, 2], bf16, isOutput=False)
    out = nc.declare_dram_parameter("out", [s, D], bf16, isOutput=True)

    with tile.TileContext(nc) as tc:
        with tc.tile_pool(name="pw", bufs=1) as pw, \
             tc.tile_pool(name="px", bufs=2) as px, \
             tc.tile_pool(name="pq", bufs=2) as pq, \
             tc.tile_pool(name="pr", bufs=2) as pr, \
             tc.tile_pool(name="pe", bufs=4) as pe_pool, \
             tc.tile_pool(name="pep", bufs=4) as pep, \
             tc.tile_pool(name="pat", bufs=8) as pat, \
             tc.tile_pool(name="pst", bufs=2) as pst, \
             tc.tile_pool(name="pao", bufs=2) as pao, \
             tc.tile_pool(name="pos", bufs=3) as pos, \
             tc.tile_pool(name="pps", bufs=1, space="PSUM") as pps:

            # ---------------- persistent SBUF ---------------------------
            # interleave chunk-0 x loads with wqk so the first projection
            # matmuls can start within a few microseconds
            qs = [nc.sync, nc.gpsimd, nc.scalar]
            wqk_sb = pw.tile([128, KO, 8 * 128], bf16)
            xt0 = px.tile([128, KO, 512], bf16, name="xt0", tag="xt")
            for ko in range(KO):
                qs[ko % 3].dma_start(xt0[:, ko, :], xT[ds(ko * 128, 128),
                                                       ts(0, 512)])
                qs[(ko + 1) % 3].dma_start(wqk_sb[:, ko, :],
                                           wqk[ds(ko * 128, 128), :])
            cs_sb = pw.tile([128, s], bf16)
            nc.scalar.dma_start(cs_sb[:], cs[:])
            sn_sb = pw.tile([128, s], bf16)
            nc.sync.dma_start(sn_sb[:], sn[:])
            wv_sb = pw.tile([128, KO, HPC * DH], bf16)
            for ko in range(KO):
                qs[ko % 3].dma_start(wv_sb[:, ko, :], wv[ds(ko * 128, 128), :])
            dg_sb = pw.tile([128, 256], f32)
            nc.sync.dma_start(dg_sb[:], dg[:])
            dgv = dg_sb[:].rearrange("p (m c) -> p m c", m=2)
            lam_sb = pw.tile([128, 2], bf16)
            nc.gpsimd.dma_start(lam_sb[:], lamv[:])

            k_sb = pw.tile([64, 2, HPC, s], bf16)
            v_sb = pw.tile([128, KT, HPC, DH + 2], bf16)
            # denominator columns: col 128 = 1.0, col 129 = 1/lambda
            nc.gpsimd.dma_start(
                out=v_sb[:, :, :, DH:DH + 2].rearrange("p a b c -> p (a b) c"),
                in_=lam_sb[:].unsqueeze(1).to_broadcast([128, KT * HPC, 2]))
            wo_sb = pw.tile([128, HPC, D], bf16)
            for hh in range(HPC):
                qs[hh % 3].dma_start(wo_sb[:, hh, :], wo[ds(hh * DH, DH), :])

            qtiles = {}

            # ---------------- projection + rope for one 512-chunk -------
            def emit_proj(ncI):
                if ncI == 0:
                    xt = xt0
                else:
                    xt = px.tile([128, KO, 512], bf16, name=f"xt{ncI}", tag="xt")
                    for ko in range(KO):
                        eng = nc.sync if ko % 2 == 0 else nc.gpsimd
                        eng.dma_start(xt[:, ko, :],
                                      xT[ds(ko * 128, 128), ts(ncI, 512)])
                q_sb = pq.tile([64, 2, HPC, 512], bf16, name=f"q{ncI}", tag="q")
                qtiles[ncI] = q_sb
                csc = cs_sb[:, ts(ncI, 512)]
                snc = sn_sb[:, ts(ncI, 512)]
                for pair in (2, 3, 0, 1):       # k maps first, v between, q last
                    if pair == 0:
                        emit_v(ncI, xt)
                    be, bo = 2 * pair, 2 * pair + 1
                    pe_ = pps.tile([128, 512], f32, tag="proj", bufs=2,
                                   name=f"pe{ncI}{pair}")
                    po_ = pps.tile([128, 512], f32, tag="proj", bufs=2,
                                   name=f"po{ncI}{pair}")
                    for ko in range(KO):
                        nc.tensor.matmul(pe_[:], wqk_sb[:, ko, ts(be, 128)],
                                         xt[:, ko, :], start=(ko == 0),
                                         stop=(ko == KO - 1))
                    for ko in range(KO):
                        nc.tensor.matmul(po_[:], wqk_sb[:, ko, ts(bo, 128)],
                                         xt[:, ko, :], start=(ko == 0),
                                         stop=(ko == KO - 1))
                    peb = pr.tile([128, 512], bf16, tag="peb")
                    pob = pr.tile([128, 512], bf16, tag="pob")
                    nc.scalar.activation(peb[:], pe_[:], AF.Copy)
                    nc.vector.tensor_copy(out=pob[:], in_=po_[:])
                    t1 = pr.tile([128, 512], bf16, tag="t1")
                    t2 = pr.tile([128, 512], bf16, tag="t2")
                    oe = pr.tile([128, 512], bf16, tag="oe")
                    nc.vector.tensor_tensor(t1[:], peb[:], csc, OP.mult)
                    nc.vector.tensor_tensor(t2[:], pob[:], snc, OP.mult)
                    nc.vector.tensor_tensor(oe[:], t1[:], t2[:], OP.subtract)
                    t3 = pr.tile([128, 512], bf16, tag="t3")
                    t4 = pr.tile([128, 512], bf16, tag="t4")
                    oo = pr.tile([128, 512], bf16, tag="oo")
                    nc.gpsimd.tensor_tensor(t3[:], peb[:], snc, OP.mult)
                    nc.gpsimd.tensor_tensor(t4[:], pob[:], csc, OP.mult)
                    nc.gpsimd.tensor_tensor(oo[:], t3[:], t4[:], OP.add)
                    # scatter into per-head layout [m1e m1o m2e m2o]
                    m = pair % 2
                    if pair >= 2:
                        for hh in range(HPC):
                            nc.sync.dma_start(
                                k_sb[ds(0, 32), m, hh, ts(ncI, 512)],
                                oe[ds(hh * 32, 32), :])
                            nc.gpsimd.dma_start(
                                k_sb[ds(32, 32), m, hh, ts(ncI, 512)],
                                oo[ds(hh * 32, 32), :])
                    else:
                        for hh in range(HPC):
                            nc.sync.dma_start(
                                q_sb[ds(0, 32), m, hh, :],
                                oe[ds(hh * 32, 32), :])
                            nc.gpsimd.dma_start(
                                q_sb[ds(32, 32), m, hh, :],
                                oo[ds(hh * 32, 32), :])

            def emit_v(ncI, xt):
                for tsub in range(4):
                    pv = pps.tile([128, HPC * DH], f32, tag="proj", bufs=2,
                                  name=f"pv{ncI}{tsub}")
                    for ko in range(KO):
                        nc.tensor.matmul(pv[:], xt[:, ko, ts(tsub, 128)],
                                         wv_sb[:, ko, :], start=(ko == 0),
                                         stop=(ko == KO - 1))
                    kt = ncI * 4 + tsub
                    nc.any.tensor_copy(
                        out=v_sb[:, kt, :, 0:DH],
                        in_=pv[:].rearrange("p (h d) -> p h d", h=HPC))

            # ---------------- attention for one (head, 256-chunk) -------
            pend = {}   # (h, qc, ki) -> e12, cross-head score warm-start

            def emit_scores(h, qc, ki):
                qoff = 256 * (qc % 2)
                q_sb = qtiles[qc // 2]
                c0 = 128 if ki == 2 * qc + 1 else 0
                s12 = pps.tile([128, 512], f32, tag="sc", bufs=2,
                               name=f"s12_{h}_{qc}_{ki}")
                for m in (0, 1):
                    nc.tensor.matmul(
                        s12[:, ds(256 * m + c0, 256 - c0)],
                        k_sb[:, m, h, ts(ki, 128)],
                        q_sb[:, m, h, ds(qoff + c0, 256 - c0)],
                        start=(m == 0), stop=True, skip_group_check=True)
                s12v = s12[:].rearrange("p (m c) -> p m c", m=2)
                import os as _os
                if _os.environ.get("KBISECT", "full") != "attn_nomask":
                    if ki == 2 * qc:        # diagonal block of subtile 0
                        nc.vector.tensor_tensor(s12v[:, :, 0:128],
                                                s12v[:, :, 0:128], dgv, OP.add)
                    elif ki == 2 * qc + 1:  # diagonal block of subtile 1
                        nc.vector.tensor_tensor(s12v[:, :, 128:256],
                                                s12v[:, :, 128:256], dgv, OP.add)
                e12 = pe_pool.tile([128, 512], bf16, tag="e",
                                   name=f"e12_{h}_{qc}_{ki}")
                e12v = e12[:].rearrange("p (m c) -> p m c", m=2)
                nc.scalar.activation(e12v[:, :, c0:256], s12v[:, :, c0:256],
                                     AF.Exp, scale=SCALE)
                return e12

            def emit_attn(h, qc, sA, sS, attp):
                klim = 2 * qc + 2

                U = [[None, None], [None, None]]
                for j in (0, 1):
                    for m in (0, 1):
                        U[j][m] = pps.tile([128, DH + 2], f32, tag=f"u{j}{m}",
                                           name=f"U{h}{qc}{j}{m}")

                if (h, qc, 0) not in pend:
                    pend[(h, qc, 0)] = emit_scores(h, qc, 0)
                for ki in range(klim):
                    if ki + 1 < klim and (h, qc, ki + 1) not in pend:
                        pend[(h, qc, ki + 1)] = emit_scores(h, qc, ki + 1)
                    e12 = pend.pop((h, qc, ki))
                    for j in (0, 1):
                        lim = 2 * qc + j
                        if ki > lim:
                            continue
                        for m in (0, 1):
                            nc.tensor.matmul(
                                U[j][m][:, 0:DH + 1 + m],
                                e12[:, ds(256 * m + 128 * j, 128)],
                                v_sb[:, ki, h, 0:DH + 1 + m],
                                start=(ki == 0), stop=(ki == lim))
                # warm-start the next (head, chunk)'s first score tiles so
                # its AV never waits on a cold exp
                if h + 1 < HPC:
                    nh, nqc = h + 1, qc
                elif qc % 2 == 0:
                    nh, nqc = 0, qc + 1     # same 512-chunk, q already there
                else:
                    nh = None
                if nh is not None:
                    pend[(nh, nqc, 0)] = emit_scores(nh, nqc, 0)
                    pend[(nh, nqc, 1)] = emit_scores(nh, nqc, 1)

                # epilogue: normalize by denominators, LN partial sums
                import os as _os
                if _os.environ.get("KBISECT", "full") == "attn_noep":
                    return
                for j in (0, 1):
                    col = 2 * h + j
                    r1 = pep.tile([128, 1], f32, tag="r1")
                    r2 = pep.tile([128, 1], f32, tag="r2")
                    nc.vector.reciprocal(out=r1[:], in_=U[j][0][:, DH:DH + 1])
                    nc.vector.reciprocal(out=r2[:], in_=U[j][1][:, DH + 1:DH + 2])
                    t2s = pep.tile([128, 128], f32, tag="t2s")
                    nc.vector.tensor_scalar(out=t2s[:], in0=U[j][1][:, 0:DH],
                                            scalar1=r2[:], scalar2=None,
                                            op0=OP.mult)
                    ap = attp[2 * h + j]
                    nc.vector.scalar_tensor_tensor(
                        out=ap[:], in0=U[j][0][:, 0:DH], scalar=r1[:],
                        in1=t2s[:], op0=OP.mult, op1=OP.subtract,
                        accum_out=sA[:, col:col + 1])
                    sqs = pep.tile([128, 128], bf16, tag="sqs")
                    nc.vector.scalar_tensor_tensor(
                        out=sqs[:], in0=ap[:], scalar=1.0, in1=ap[:],
                        op0=OP.mult, op1=OP.mult,
                        accum_out=sS[:, col:col + 1])

            # ---------------- LN finalize + transpose for one 256-chunk -
            def emit_norm(qc, sA, sS, attp, attnT):
                nmu = pst.tile([128, 8], f32, tag="nmu")
                ex2 = pst.tile([128, 8], f32, tag="ex2")
                nc.vector.tensor_scalar(out=nmu[:], in0=sA[:], scalar1=-1.0 / DH,
                                        scalar2=None, op0=OP.mult)
                nc.vector.tensor_scalar(out=ex2[:], in0=sS[:], scalar1=1.0 / DH,
                                        scalar2=None, op0=OP.mult)
                msq = pst.tile([128, 8], f32, tag="msq")
                nc.vector.tensor_tensor(msq[:], nmu[:], nmu[:], OP.mult)
                var = pst.tile([128, 8], f32, tag="var")
                nc.vector.tensor_tensor(var[:], ex2[:], msq[:], OP.subtract)
                # rsqrt(var + eps) via Quake bit-trick + 2 Newton steps
                # (keeps the whole LN finalize off ACT: no act-table thrash)
                vps = pst.tile([128, 8], f32, tag="vps")
                nc.vector.tensor_scalar(out=vps[:], in0=var[:], scalar1=1e-5,
                                        scalar2=None, op0=OP.add)
                i32 = mybir.dt.int32
                ysh = pst.tile([128, 8], f32, tag="ysh")
                nc.vector.tensor_scalar(out=ysh[:].bitcast(i32),
                                        in0=vps[:].bitcast(i32), scalar1=1,
                                        scalar2=None, op0=OP.arith_shift_right)
                y0 = pst.tile([128, 8], f32, tag="y0")
                nc.vector.tensor_scalar(out=y0[:].bitcast(i32),
                                        in0=ysh[:].bitcast(i32), scalar1=-1,
                                        scalar2=0x5f3759df, op0=OP.mult,
                                        op1=OP.add)
                rstd = y0
                for it in range(2):
                    yy = pst.tile([128, 8], f32, tag=f"yy{it}")
                    nc.vector.tensor_tensor(yy[:], rstd[:], rstd[:], OP.mult)
                    yv = pst.tile([128, 8], f32, tag=f"yv{it}")
                    nc.vector.tensor_tensor(yv[:], yy[:], vps[:], OP.mult)
                    yc = pst.tile([128, 8], f32, tag=f"yc{it}")
                    nc.vector.tensor_scalar(out=yc[:], in0=yv[:], scalar1=-0.5,
                                            scalar2=1.5, op0=OP.mult, op1=OP.add)
                    yn = pst.tile([128, 8], f32, tag=f"yn{it}")
                    nc.vector.tensor_tensor(yn[:], rstd[:], yc[:], OP.mult)
                    rstd = yn
                nbias = pst.tile([128, 8], f32, tag="nbias")
                nc.vector.tensor_tensor(nbias[:], nmu[:], rstd[:], OP.mult)
                for h in range(HPC):
                    for j in (0, 1):
                        col = 2 * h + j
                        attn = pat.tile([128, 128], bf16, tag="attn",
                                        name=f"attn{qc}{h}{j}")
                        nc.any.tensor_scalar(out=attn[:], in0=attp[col][:],
                                             scalar1=rstd[:, col:col + 1],
                                             scalar2=nbias[:, col:col + 1],
                                             op0=OP.mult, op1=OP.add)
                        nc.sync.dma_start_transpose(
                            attnT[:, h, ds(128 * j, 128)], attn[:])

            # ---------------- W_o partial for one 256-chunk -------------
            def emit_wo_piece(qc, attnT, piece):
                # piece in 0..3, two (qi, nj) groups each
                for g in (2 * piece, 2 * piece + 1):
                    qi, nj = g // 4, g % 4
                    po = pps.tile([128, 512], f32, tag="proj", bufs=2,
                                  name=f"wo{qc}{qi}{nj}")
                    for hh in range(HPC):
                        nc.tensor.matmul(po[:],
                                         attnT[:, hh, ds(128 * qi, 128)],
                                         wo_sb[:, hh, ts(nj, 512)],
                                         start=(hh == 0),
                                         stop=(hh == HPC - 1))
                    ost = pos.tile([128, 512], bf16, tag="ost")
                    nc.any.tensor_copy(out=ost[:], in_=po[:])
                    eng = nc.sync if nj % 2 == 0 else nc.gpsimd
                    eng.dma_start(out[ds(qc * 256 + qi * 128, 128),
                                      ts(nj, 512)], ost[:])

            # ---------------- main fused loop ---------------------------
            attnTs = {}
            emit_proj(0)
            for ncI in range(NCH):
                for half in (0, 1):
                    qc = 2 * ncI + half
                    if BISECT == "attn_half" and qc >= 4:
                        break
                    if BISECT in ("proj",):
                        if half == 0 and ncI + 1 < NCH:
                            emit_proj(ncI + 1)
                        continue
                    sA = pst.tile([128, 8], f32, tag="sA", name=f"sA{qc}")
                    sS = pst.tile([128, 8], f32, tag="sS", name=f"sS{qc}")
                    attp = [pat.tile([128, 128], f32, tag=f"attp{i}",
                                     name=f"attp{qc}_{i}", bufs=1)
                            for i in range(8)]
                    for h in range(HPC):
                        if BISECT == "attn_one" and (qc != 0 or h != 0):
                            continue
                        emit_attn(h, qc, sA, sS, attp)
                        # W_o of the previous chunk, sprinkled between heads
                        # so PE has ready filler at every head boundary
                        if qc >= 1 and BISECT == "full":
                            emit_wo_piece(qc - 1, attnTs[qc - 1], h)
                    if qc >= 1 and BISECT == "full":
                        attnTs.pop(qc - 1)
                    if BISECT in ("norm", "full"):
                        attnT = pao.tile([128, HPC, 256], bf16,
                                         name=f"attnT{qc}", tag="attnT")
                        attnTs[qc] = attnT
                        emit_norm(qc, sA, sS, attp, attnT)
                    # next chunk's projection between the two halves, so the
                    # scheduler has dense PE work during attention phases
                    if half == 0 and ncI + 1 < NCH:
                        emit_proj(ncI + 1)
            if BISECT == "full":
                for piece in range(4):
                    emit_wo_piece(NQC - 1, attnTs[NQC - 1], piece)
            if BISECT != "full":
                dwr = pos.tile([128, 512], bf16, tag="ost")
                nc.gpsimd.memset(dwr[:], 0.0)
                nc.sync.dma_start(out[ds(0, 128), ts(0, 512)], dwr[:])

    nc.finalize()
    return nc


def get_program(s=S):
    if s not in _PROGRAM_CACHE:
        _PROGRAM_CACHE[s] = build_program(s)
    return _PROGRAM_CACHE[s]


def make_core_inputs(x, cos, sin, W_qkv, W_o, ln_gamma, lbda, core, s=S):
    """Host-side shard prep for one core."""
    b, hg = core // 4, core % 4
    heads = list(range(hg * HPC, (hg + 1) * HPC))

    def qk_block_cols(base, dstart):
        # even/odd pair columns for one 32-wide block across the 4 heads
        return [base + hh * DH + dstart + 2 * p for hh in heads for p in range(32)]

    cols = []
    for base in (0, D):                       # q section, k section
        for dstart in (0, 1, HD, HD + 1):     # m1-even, m1-odd, m2-even, m2-odd
            cols += qk_block_cols(base, dstart)
    wqk = np.ascontiguousarray(W_qkv[:, cols]).astype(bf16_np)
    vcols = [2 * D + hh * DH + dd for hh in heads for dd in range(DH)]
    wv = np.ascontiguousarray(W_qkv[:, vcols]).astype(bf16_np)
    worows = [hh * DH + dd for hh in heads for dd in range(DH)]
    gamma_scale = np.concatenate([ln_gamma[hh] * OUT_MULT for hh in heads])
    wo = np.ascontiguousarray(W_o[worows, :] * gamma_scale[:, None]).astype(bf16_np)

    xT = np.ascontiguousarray(x[b].T).astype(bf16_np)
    cst = np.ascontiguousarray(np.tile(cos.T, (HPC, 1))).astype(bf16_np)
    snt = np.ascontiguousarray(np.tile(sin.T, (HPC, 1))).astype(bf16_np)

    diag = np.where(np.triu(np.ones((128, 128), dtype=bool)), 0.0, NEG)
    dg2 = np.concatenate([diag, diag], axis=1).astype(np.float32)

    lam2 = np.zeros((128, 2), dtype=np.float32)
    lam2[:, 0] = 1.0
    lam2[:, 1] = 1.0 / max(float(lbda), 1e-6)

    return {
        "xT": xT, "wqk": wqk, "wv": wv, "wo": wo, "cs": cst, "sn": snt,
        "dg": dg2, "lamv": lam2.astype(bf16_np),
    }


def _mask_is_causal(mask, s=S):
    m = np.asarray(mask).reshape(s, s)
    tril = np.tril(np.ones((s, s), dtype=bool))
    if not np.array_equal(m == 0.0, tril):
        return False
    off = m[~tril]
    return off.size == 0 or (np.all(off <= -1.0e8) and np.all(np.isfinite(off)))


def _numpy_reference(x, mask, cos, sin, W_qkv, W_o, ln_gamma, ln_beta, lbda):
    """Exact-math fallback (used only if the mask is not the causal pattern)."""
    b, s, d = x.shape
    qkv = x @ W_qkv
    q, k, v = np.split(qkv, 3, axis=-1)
    q = q.reshape(b, s, H, DH).transpose(0, 2, 1, 3)
    k = k.reshape(b, s, H, DH).transpose(0, 2, 1, 3)
    v = v.reshape(b, s, H, DH).transpose(0, 2, 1, 3)

    def rope(t):
        tr = t.reshape(b, H, s, HD // 2, 2)
        x1, x2 = tr[..., 0], tr[..., 1]
        c = cos[None, None]
        sn_ = sin[None, None]
        o1 = x1 * c - x2 * sn_
        o2 = x1 * sn_ + x2 * c
        return np.stack([o1, o2], axis=-1).reshape(b, H, s, HD)

    q1, q2 = q[..., :HD], q[..., HD:]
    k1, k2 = k[..., :HD], k[..., HD:]
    q1, k1 = rope(q1), rope(k1)
    q2, k2 = rope(q2), rope(k2)

    def softm(z):
        z = z - z.max(-1, keepdims=True)
        e = np.exp(z)
        return e / e.sum(-1, keepdims=True)

    m = np.asarray(mask).reshape(1, 1, s, s)
    a1 = softm(np.einsum("bhqd,bhkd->bhqk", q1, k1) * SCALE + m)
    a2 = softm(np.einsum("bhqd,bhkd->bhqk", q2, k2) * SCALE + m)
    a = a1 - float(lbda) * a2
    o = np.einsum("bhqk,bhkd->bhqd", a, v)
    mu = o.mean(-1, keepdims=True)
    var = o.var(-1, keepdims=True)
    o = (o - mu) / np.sqrt(var + 1e-5)
    o = o * ln_gamma[None, :, None, :] + ln_beta[None, :, None, :]
    o = o * OUT_MULT
    o = o.transpose(0, 2, 1, 3).reshape(b, s, d)
    return (o @ W_o).astype(np.float32)


def kernel(x, mask, cos, sin, W_qkv, W_o, ln_gamma, ln_beta, lbda):
    global LAST_RESULTS, LAST_EXEC_NS
    x = np.asarray(x, dtype=np.float32)
    cos = np.asarray(cos, dtype=np.float32)
    sin = np.asarray(sin, dtype=np.float32)
    W_qkv = np.asarray(W_qkv, dtype=np.float32)
    W_o = np.asarray(W_o, dtype=np.float32)
    ln_gamma = np.asarray(ln_gamma, dtype=np.float32)
    ln_beta = np.asarray(ln_beta, dtype=np.float32)
    lbda_f = float(np.asarray(lbda))

    if not _mask_is_causal(mask):
        return _numpy_reference(x, mask, cos, sin, W_qkv, W_o,
                                ln_gamma, ln_beta, lbda_f)

    from concourse.bass_utils import run_bass_kernel_spmd

    nc = get_program(S)
    in_maps = [
        make_core_inputs(x, cos, sin, W_qkv, W_o, ln_gamma, lbda_f, c)
        for c in range(NCORES)
    ]
    kwargs = {"trace": TRACE}
    if TRACE and TRACE_DIR:
        kwargs["tmpdir"] = TRACE_DIR
    res = run_bass_kernel_spmd(nc, in_maps, core_ids=list(range(NCORES)),
                               **kwargs)
    LAST_RESULTS = res
    LAST_EXEC_NS = getattr(res, "exec_time_ns", None)

    outf = np.zeros((B, S, D), dtype=np.float32)
    for c in range(NCORES):
        outf[c // 4] += res.results[c]["out"].astype(np.float32)
    # ln_beta rank-1 term: (beta * OUT_MULT) @ W_o added to every token
    beta_term = (ln_beta.reshape(-1) * OUT_MULT) @ W_o
    outf += beta_term[None, None, :]
    return outf


# revision 30
# speedup vs baseline: 1.3423x; 1.0012x over previous
"""Differential attention (B=2, S=2048, D=2048, H=16) on 8 Trainium2 cores.

Sharding: core c -> batch b=c//4, head group hg=c%4 (4 heads each).
Fully fused single-pass kernel, all matmuls in bf16:
  per 512-token chunk: qkv projection -> RoPE (bf16 elementwise on DVE/Pool)
  -> scatter into per-head SBUF K/V/Q (no DRAM roundtrip); then per
  256-token attention chunk: scores [k,q] -> exp (one ACT op spans both
  maps) -> flipped AV (stationary = exp-block, moving = v with appended
  ones / (1/lambda) columns) accumulating U[q, vd+denom] in PSUM, so the
  softmax denominators ride free in columns 128/129 -> LN stats via DVE
  accum_out -> normalize (ACT per-partition scale/bias) -> DMA-transpose
  -> partial @ W_o interleaved one chunk behind as PE filler.
ln_gamma * (1-LBDA_INIT) is folded into W_o rows host-side; the ln_beta
rank-1 term is added on the host after the gather.
"""
import sys

sys.path.insert(0, "/opt/trn_rl_repo")

import numpy as np
import ml_dtypes

B, S, D = 2, 2048, 2048
H = 16
HD = D // (2 * H)          # 64 per-map head dim
DH = 2 * HD                # 128 per-head dim
HPC = H // 4               # 4 heads per core
NCORES = 8
SCALE = HD ** -0.5         # 0.125
NEG = -8.0e9               # mask add value pre-scale (-1e9 / SCALE)
OUT_MULT = 1.0 - 0.8       # (1 - LBDA_INIT)

NCH = S // 512             # 4 projection chunks
NQC = S // 256             # 8 attention chunks
KT = S // 128              # 16 k tiles
KO = D // 128              # 16 contraction chunks

# module-level knobs / results for test.py
TRACE = False
TRACE_DIR = None
LAST_RESULTS = None
LAST_EXEC_NS = None

_PROGRAM_CACHE = {}

bf16_np = ml_dtypes.bfloat16


def build_program(s=S):
    """Build the per-core Bass program (SPMD: same program, 8 cores)."""
    import os
    BISECT = os.environ.get("KBISECT", "full")
    import concourse.bass as bass
    import concourse.tile as tile
    from concourse import bacc, mybir
    from concourse.bass import ts, ds

    f32 = mybir.dt.float32
    bf16 = mybir.dt.bfloat16
    AF = mybir.ActivationFunctionType
    OP = mybir.AluOpType

    nc = bacc.Bacc()
    xT = nc.declare_dram_parameter("xT", [D, s], bf16, isOutput=False)
    wqk = nc.declare_dram_parameter("wqk", [D, 8 * 128], bf16, isOutput=False)
    wv = nc.declare_dram_parameter("wv", [D, HPC * DH], bf16, isOutput=False)
    wo = nc.declare_dram_parameter("wo", [HPC * DH, D], bf16, isOutput=False)
    cs = nc.declare_dram_parameter("cs", [128, s], bf16, isOutput=False)
    sn = nc.declare_dram_parameter("sn", [128, s], bf16, isOutput=False)
    dg = nc.declare_dram_parameter("dg", [128, 256], f32, isOutput=False)
    lamv = nc.declare_dram_parameter("lamv", [128, 2], bf16, isOutput=False)
    out = nc.declare_dram_parameter("out", [s, D], bf16, isOutput=True)

    with tile.TileContext(nc) as tc:
        with tc.tile_pool(name="pw", bufs=1) as pw, \
             tc.tile_pool(name="px", bufs=2) as px, \
             tc.tile_pool(name="pq", bufs=2) as pq, \
             tc.tile_pool(name="pr", bufs=2) as pr, \
             tc.tile_pool(name="pe", bufs=4) as pe_pool, \
             tc.tile_pool(name="pep", bufs=4) as pep, \
             tc.tile_pool(name="pat", bufs=8) as pat, \
             tc.tile_pool(name="pst", bufs=2) as pst, \
             tc.tile_pool(name="pao", bufs=2) as pao, \
             tc.tile_pool(name="pos", bufs=3) as pos, \
             tc.tile_pool(name="pps", bufs=1, space="PSUM") as pps:

            # ---------------- persistent SBUF ---------------------------
            # interleave chunk-0 x loads with wqk so the first projection
            # matmuls can start within a few microseconds
            qs = [nc.sync, nc.gpsimd, nc.scalar]
            wqk_sb = pw.tile([128, KO, 8 * 128], bf16)
            xt0 = px.tile([128, KO, 512], bf16, name="xt0", tag="xt")
            for ko in range(KO):
                qs[ko % 3].dma_start(xt0[:, ko, :], xT[ds(ko * 128, 128),
                                                       ts(0, 512)])
                qs[(ko + 1) % 3].dma_start(wqk_sb[:, ko, :],
                                           wqk[ds(ko * 128, 128), :])
            cs_sb = pw.tile([128, s], bf16)
            nc.scalar.dma_start(cs_sb[:], cs[:])
            sn_sb = pw.tile([128, s], bf16)
            nc.sync.dma_start(sn_sb[:], sn[:])
            wv_sb = pw.tile([128, KO, HPC * DH], bf16)
            for ko in range(KO):
                qs[ko % 3].dma_start(wv_sb[:, ko, :], wv[ds(ko * 128, 128), :])
            dg_sb = pw.tile([128, 256], f32)
            nc.sync.dma_start(dg_sb[:], dg[:])
            dgv = dg_sb[:].rearrange("p (m c) -> p m c", m=2)
            lam_sb = pw.tile([128, 2], bf16)
            nc.gpsimd.dma_start(lam_sb[:], lamv[:])

            k_sb = pw.tile([64, 2, HPC, s], bf16)
            v_sb = pw.tile([128, KT, HPC, DH + 2], bf16)
            # denominator columns: col 128 = 1.0, col 129 = 1/lambda
            nc.gpsimd.dma_start(
                out=v_sb[:, :, :, DH:DH + 2].rearrange("p a b c -> p (a b) c"),
                in_=lam_sb[:].unsqueeze(1).to_broadcast([128, KT * HPC, 2]))
            wo_sb = pw.tile([128, HPC, D], bf16)
            for hh in range(HPC):
                qs[hh % 3].dma_start(wo_sb[:, hh, :], wo[ds(hh * DH, DH), :])

            qtiles = {}

            # ---------------- projection + rope for one 512-chunk -------
            def emit_proj(ncI):
                if ncI == 0:
                    xt = xt0
                else:
                    xt = px.tile([128, KO, 512], bf16, name=f"xt{ncI}", tag="xt")
                    for ko in range(KO):
                        eng = nc.sync if ko % 2 == 0 else nc.gpsimd
                        eng.dma_start(xt[:, ko, :],
                                      xT[ds(ko * 128, 128), ts(ncI, 512)])
                q_sb = pq.tile([64, 2, HPC, 512], bf16, name=f"q{ncI}", tag="q")
                qtiles[ncI] = q_sb
                csc = cs_sb[:, ts(ncI, 512)]
                snc = sn_sb[:, ts(ncI, 512)]
                for pair in (2, 3, 0, 1):       # k maps first, v between, q last
                    if pair == 0:
                        emit_v(ncI, xt)
                    be, bo = 2 * pair, 2 * pair + 1
                    pe_ = pps.tile([128, 512], f32, tag="proj", bufs=2,
                                   name=f"pe{ncI}{pair}")
                    po_ = pps.tile([128, 512], f32, tag="proj", bufs=2,
                                   name=f"po{ncI}{pair}")
                    for ko in range(KO):
                        nc.tensor.matmul(pe_[:], wqk_sb[:, ko, ts(be, 128)],
                                         xt[:, ko, :], start=(ko == 0),
                                         stop=(ko == KO - 1))
                    for ko in range(KO):
                        nc.tensor.matmul(po_[:], wqk_sb[:, ko, ts(bo, 128)],
                                         xt[:, ko, :], start=(ko == 0),
                                         stop=(ko == KO - 1))
                    peb = pr.tile([128, 512], bf16, tag="peb")
                    pob = pr.tile([128, 512], bf16, tag="pob")
                    nc.scalar.activation(peb[:], pe_[:], AF.Copy)
                    nc.vector.tensor_copy(out=pob[:], in_=po_[:])
                    t1 = pr.tile([128, 512], bf16, tag="t1")
                    t2 = pr.tile([128, 512], bf16, tag="t2")
                    oe = pr.tile([128, 512], bf16, tag="oe")
                    nc.vector.tensor_tensor(t1[:], peb[:], csc, OP.mult)
                    nc.vector.tensor_tensor(t2[:], pob[:], snc, OP.mult)
                    nc.vector.tensor_tensor(oe[:], t1[:], t2[:], OP.subtract)
                    t3 = pr.tile([128, 512], bf16, tag="t3")
                    t4 = pr.tile([128, 512], bf16, tag="t4")
                    oo = pr.tile([128, 512], bf16, tag="oo")
                    nc.gpsimd.tensor_tensor(t3[:], peb[:], snc, OP.mult)
                    nc.gpsimd.tensor_tensor(t4[:], pob[:], csc, OP.mult)
                    nc.gpsimd.tensor_tensor(oo[:], t3[:], t4[:], OP.add)
                    # scatter into per-head layout [m1e m1o m2e m2o]
                    m = pair % 2
                    if pair >= 2:
                        for hh in range(HPC):
                            nc.sync.dma_start(
                                k_sb[ds(0, 32), m, hh, ts(ncI, 512)],
                                oe[ds(hh * 32, 32), :])
                            nc.gpsimd.dma_start(
                                k_sb[ds(32, 32), m, hh, ts(ncI, 512)],
                                oo[ds(hh * 32, 32), :])
                    else:
                        for hh in range(HPC):
                            nc.sync.dma_start(
                                q_sb[ds(0, 32), m, hh, :],
                                oe[ds(hh * 32, 32), :])
                            nc.gpsimd.dma_start(
                                q_sb[ds(32, 32), m, hh, :],
                                oo[ds(hh * 32, 32), :])

            def emit_v(ncI, xt):
                for tsub in range(4):
                    pv = pps.tile([128, HPC * DH], f32, tag="proj", bufs=2,
                                  name=f"pv{ncI}{tsub}")
                    for ko in range(KO):
                        nc.tensor.matmul(pv[:], xt[:, ko, ts(tsub, 128)],
                                         wv_sb[:, ko, :], start=(ko == 0),
                                         stop=(ko == KO - 1))
                    kt = ncI * 4 + tsub
                    nc.any.tensor_copy(
                        out=v_sb[:, kt, :, 0:DH],
                        in_=pv[:].rearrange("p (h d) -> p h d", h=HPC))

            # ---------------- attention for one (head, 256-chunk) -------
            pend = {}   # (h, qc, ki) -> e12, cross-head score warm-start

            def emit_scores(h, qc, ki):
                qoff = 256 * (qc % 2)
                q_sb = qtiles[qc // 2]
                c0 = 128 if ki == 2 * qc + 1 else 0
                s12 = pps.tile([128, 512], f32, tag="sc", bufs=2,
                               name=f"s12_{h}_{qc}_{ki}")
                for m in (0, 1):
                    nc.tensor.matmul(
                        s12[:, ds(256 * m + c0, 256 - c0)],
                        k_sb[:, m, h, ts(ki, 128)],
                        q_sb[:, m, h, ds(qoff + c0, 256 - c0)],
                        start=(m == 0), stop=True, skip_group_check=True)
                s12v = s12[:].rearrange("p (m c) -> p m c", m=2)
                import os as _os
                if _os.environ.get("KBISECT", "full") != "attn_nomask":
                    if ki == 2 * qc:        # diagonal block of subtile 0
                        nc.vector.tensor_tensor(s12v[:, :, 0:128],
                                                s12v[:, :, 0:128], dgv, OP.add)
                    elif ki == 2 * qc + 1:  # diagonal block of subtile 1
                        nc.vector.tensor_tensor(s12v[:, :, 128:256],
                                                s12v[:, :, 128:256], dgv, OP.add)
                e12 = pe_pool.tile([128, 512], bf16, tag="e",
                                   name=f"e12_{h}_{qc}_{ki}")
                e12v = e12[:].rearrange("p (m c) -> p m c", m=2)
                nc.scalar.activation(e12v[:, :, c0:256], s12v[:, :, c0:256],
                                     AF.Exp, scale=SCALE)
                return e12

            def emit_attn(h, qc, sA, sS, attp):
                klim = 2 * qc + 2

                U = [[None, None], [None, None]]
                for j in (0, 1):
                    for m in (0, 1):
                        U[j][m] = pps.tile([128, DH + 2], f32, tag=f"u{j}{m}",
                                           name=f"U{h}{qc}{j}{m}")

                if (h, qc, 0) not in pend:
                    pend[(h, qc, 0)] = emit_scores(h, qc, 0)
                for ki in range(klim):
                    if ki + 1 < klim and (h, qc, ki + 1) not in pend:
                        pend[(h, qc, ki + 1)] = emit_scores(h, qc, ki + 1)
                    e12 = pend.pop((h, qc, ki))
                    for j in (0, 1):
                        lim = 2 * qc + j
                        if ki > lim:
                            continue
                        for m in (0, 1):
                            nc.tensor.matmul(
                                U[j][m][:, 0:DH + 1 + m],
                                e12[:, ds(256 * m + 128 * j, 128)],
                                v_sb[:, ki, h, 0:DH + 1 + m],
                                start=(ki == 0), stop=(ki == lim))
                # warm-start the next (head, chunk)'s first score tiles so
                # its AV never waits on a cold exp
                if h + 1 < HPC:
                    nh, nqc = h + 1, qc
                elif qc + 1 < NQC:
                    nh, nqc = 0, qc + 1     # next chunk's proj already emitted
                else:
                    nh = None
                if nh is not None:
                    pend[(nh, nqc, 0)] = emit_scores(nh, nqc, 0)
                    pend[(nh, nqc, 1)] = emit_scores(nh, nqc, 1)

                # epilogue: normalize by denominators, LN partial sums
                import os as _os
                if _os.environ.get("KBISECT", "full") == "attn_noep":
                    return
                for j in (0, 1):
                    col = 2 * h + j
                    r1 = pep.tile([128, 1], f32, tag="r1")
                    r2 = pep.tile([128, 1], f32, tag="r2")
                    nc.vector.reciprocal(out=r1[:], in_=U[j][0][:, DH:DH + 1])
                    nc.vector.reciprocal(out=r2[:], in_=U[j][1][:, DH + 1:DH + 2])
                    t2s = pep.tile([128, 128], f32, tag="t2s")
                    nc.vector.tensor_scalar(out=t2s[:], in0=U[j][1][:, 0:DH],
                                            scalar1=r2[:], scalar2=None,
                                            op0=OP.mult)
                    ap = attp[2 * h + j]
                    nc.vector.scalar_tensor_tensor(
                        out=ap[:], in0=U[j][0][:, 0:DH], scalar=r1[:],
                        in1=t2s[:], op0=OP.mult, op1=OP.subtract,
                        accum_out=sA[:, col:col + 1])
                    sqs = pep.tile([128, 128], bf16, tag="sqs")
                    nc.vector.scalar_tensor_tensor(
                        out=sqs[:], in0=ap[:], scalar=1.0, in1=ap[:],
                        op0=OP.mult, op1=OP.mult,
                        accum_out=sS[:, col:col + 1])

            # ---------------- LN finalize + transpose for one 256-chunk -
            def emit_norm(qc, sA, sS, attp, attnT):
                nmu = pst.tile([128, 8], f32, tag="nmu")
                ex2 = pst.tile([128, 8], f32, tag="ex2")
                nc.vector.tensor_scalar(out=nmu[:], in0=sA[:], scalar1=-1.0 / DH,
                                        scalar2=None, op0=OP.mult)
                nc.vector.tensor_scalar(out=ex2[:], in0=sS[:], scalar1=1.0 / DH,
                                        scalar2=None, op0=OP.mult)
                msq = pst.tile([128, 8], f32, tag="msq")
                nc.vector.tensor_tensor(msq[:], nmu[:], nmu[:], OP.mult)
                var = pst.tile([128, 8], f32, tag="var")
                nc.vector.tensor_tensor(var[:], ex2[:], msq[:], OP.subtract)
                # rsqrt(var + eps) via Quake bit-trick + 2 Newton steps
                # (keeps the whole LN finalize off ACT: no act-table thrash)
                vps = pst.tile([128, 8], f32, tag="vps")
                nc.vector.tensor_scalar(out=vps[:], in0=var[:], scalar1=1e-5,
                                        scalar2=None, op0=OP.add)
                i32 = mybir.dt.int32
                ysh = pst.tile([128, 8], f32, tag="ysh")
                nc.vector.tensor_scalar(out=ysh[:].bitcast(i32),
                                        in0=vps[:].bitcast(i32), scalar1=1,
                                        scalar2=None, op0=OP.arith_shift_right)
                y0 = pst.tile([128, 8], f32, tag="y0")
                nc.vector.tensor_scalar(out=y0[:].bitcast(i32),
                                        in0=ysh[:].bitcast(i32), scalar1=-1,
                                        scalar2=0x5f3759df, op0=OP.mult,
                                        op1=OP.add)
                rstd = y0
                for it in range(2):
                    yy = pst.tile([128, 8], f32, tag=f"yy{it}")
                    nc.vector.tensor_tensor(yy[:], rstd[:], rstd[:], OP.mult)
                    yv = pst.tile([128, 8], f32, tag=f"yv{it}")
                    nc.vector.tensor_tensor(yv[:], yy[:], vps[:], OP.mult)
                    yc = pst.tile([128, 8], f32, tag=f"yc{it}")
                    nc.vector.tensor_scalar(out=yc[:], in0=yv[:], scalar1=-0.5,
                                            scalar2=1.5, op0=OP.mult, op1=OP.add)
                    yn = pst.tile([128, 8], f32, tag=f"yn{it}")
                    nc.vector.tensor_tensor(yn[:], rstd[:], yc[:], OP.mult)
                    rstd = yn
                nbias = pst.tile([128, 8], f32, tag="nbias")
                nc.vector.tensor_tensor(nbias[:], nmu[:], rstd[:], OP.mult)
                for j in (0, 1):
                    for h in range(HPC):
                        col = 2 * h + j
                        attn = pat.tile([128, 128], bf16, tag="attn",
                                        name=f"attn{qc}{h}{j}")
                        nc.any.tensor_scalar(out=attn[:], in0=attp[col][:],
                                             scalar1=rstd[:, col:col + 1],
                                             scalar2=nbias[:, col:col + 1],
                                             op0=OP.mult, op1=OP.add)
                        nc.sync.dma_start_transpose(
                            attnT[:, h, ds(128 * j, 128)], attn[:])
                    if qc == NQC - 1:
                        # last chunk: start W_o on this token-half right away
                        emit_wo_piece(qc, attnT, 2 * j)
                        emit_wo_piece(qc, attnT, 2 * j + 1)

            # ---------------- W_o partial for one 256-chunk -------------
            def emit_wo_piece(qc, attnT, piece):
                # piece in 0..3, two (qi, nj) groups each
                for g in (2 * piece, 2 * piece + 1):
                    qi, nj = g // 4, g % 4
                    po = pps.tile([128, 512], f32, tag="proj", bufs=2,
                                  name=f"wo{qc}{qi}{nj}")
                    for hh in range(HPC):
                        nc.tensor.matmul(po[:],
                                         attnT[:, hh, ds(128 * qi, 128)],
                                         wo_sb[:, hh, ts(nj, 512)],
                                         start=(hh == 0),
                                         stop=(hh == HPC - 1))
                    ost = pos.tile([128, 512], bf16, tag="ost")
                    nc.any.tensor_copy(out=ost[:], in_=po[:])
                    eng = nc.sync if nj % 2 == 0 else nc.gpsimd
                    eng.dma_start(out[ds(qc * 256 + qi * 128, 128),
                                      ts(nj, 512)], ost[:])

            # ---------------- main fused loop ---------------------------
            attnTs = {}
            emit_proj(0)
            for ncI in range(NCH):
                for half in (0, 1):
                    qc = 2 * ncI + half
                    if BISECT == "attn_half" and qc >= 4:
                        break
                    if BISECT in ("proj",):
                        if half == 0 and ncI + 1 < NCH:
                            emit_proj(ncI + 1)
                        continue
                    sA = pst.tile([128, 8], f32, tag="sA", name=f"sA{qc}")
                    sS = pst.tile([128, 8], f32, tag="sS", name=f"sS{qc}")
                    attp = [pat.tile([128, 128], f32, tag=f"attp{i}",
                                     name=f"attp{qc}_{i}", bufs=1)
                            for i in range(8)]
                    pieces = [[0], [1], [2], [3]] if qc <= 4 else \
                        [[], [0], [1, 2], [3]]
                    for h in range(HPC):
                        if BISECT == "attn_one" and (qc != 0 or h != 0):
                            continue
                        emit_attn(h, qc, sA, sS, attp)
                        # W_o of the previous chunk, sprinkled between heads
                        # so PE has ready filler at every head boundary
                        if qc >= 1 and BISECT == "full":
                            for pc in pieces[h]:
                                emit_wo_piece(qc - 1, attnTs[qc - 1], pc)
                    if qc >= 1 and BISECT == "full":
                        attnTs.pop(qc - 1)
                    if BISECT in ("norm", "full"):
                        attnT = pao.tile([128, HPC, 256], bf16,
                                         name=f"attnT{qc}", tag="attnT")
                        attnTs[qc] = attnT
                        emit_norm(qc, sA, sS, attp, attnT)
                    # next chunk's projection between the two halves, so the
                    # scheduler has dense PE work during attention phases
                    if half == 0 and ncI + 1 < NCH:
                        emit_proj(ncI + 1)

            if BISECT != "full":
                dwr = pos.tile([128, 512], bf16, tag="ost")
                nc.gpsimd.memset(dwr[:], 0.0)
                nc.sync.dma_start(out[ds(0, 128), ts(0, 512)], dwr[:])

    nc.finalize()
    return nc


def get_program(s=S):
    if s not in _PROGRAM_CACHE:
        _PROGRAM_CACHE[s] = build_program(s)
    return _PROGRAM_CACHE[s]


def make_core_inputs(x, cos, sin, W_qkv, W_o, ln_gamma, lbda, core, s=S):
    """Host-side shard prep for one core."""
    b, hg = core // 4, core % 4
    heads = list(range(hg * HPC, (hg + 1) * HPC))

    def qk_block_cols(base, dstart):
        # even/odd pair columns for one 32-wide block across the 4 heads
        return [base + hh * DH + dstart + 2 * p for hh in heads for p in range(32)]

    cols = []
    for base in (0, D):                       # q section, k section
        for dstart in (0, 1, HD, HD + 1):     # m1-even, m1-odd, m2-even, m2-odd
            cols += qk_block_cols(base, dstart)
    wqk = np.ascontiguousarray(W_qkv[:, cols]).astype(bf16_np)
    vcols = [2 * D + hh * DH + dd for hh in heads for dd in range(DH)]
    wv = np.ascontiguousarray(W_qkv[:, vcols]).astype(bf16_np)
    worows = [hh * DH + dd for hh in heads for dd in range(DH)]
    gamma_scale = np.concatenate([ln_gamma[hh] * OUT_MULT for hh in heads])
    wo = np.ascontiguousarray(W_o[worows, :] * gamma_scale[:, None]).astype(bf16_np)

    xT = np.ascontiguousarray(x[b].T).astype(bf16_np)
    cst = np.ascontiguousarray(np.tile(cos.T, (HPC, 1))).astype(bf16_np)
    snt = np.ascontiguousarray(np.tile(sin.T, (HPC, 1))).astype(bf16_np)

    diag = np.where(np.triu(np.ones((128, 128), dtype=bool)), 0.0, NEG)
    dg2 = np.concatenate([diag, diag], axis=1).astype(np.float32)

    lam2 = np.zeros((128, 2), dtype=np.float32)
    lam2[:, 0] = 1.0
    lam2[:, 1] = 1.0 / max(float(lbda), 1e-6)

    return {
        "xT": xT, "wqk": wqk, "wv": wv, "wo": wo, "cs": cst, "sn": snt,
        "dg": dg2, "lamv": lam2.astype(bf16_np),
    }


def _mask_is_causal(mask, s=S):
    m = np.asarray(mask).reshape(s, s)
    tril = np.tril(np.ones((s, s), dtype=bool))
    if not np.array_equal(m == 0.0, tril):
        return False
    off = m[~tril]
    return off.size == 0 or (np.all(off <= -1.0e8) and np.all(np.isfinite(off)))


def _numpy_reference(x, mask, cos, sin, W_qkv, W_o, ln_gamma, ln_beta, lbda):
    """Exact-math fallback (used only if the mask is not the causal pattern)."""
    b, s, d = x.shape
    qkv = x @ W_qkv
    q, k, v = np.split(qkv, 3, axis=-1)
    q = q.reshape(b, s, H, DH).transpose(0, 2, 1, 3)
    k = k.reshape(b, s, H, DH).transpose(0, 2, 1, 3)
    v = v.reshape(b, s, H, DH).transpose(0, 2, 1, 3)

    def rope(t):
        tr = t.reshape(b, H, s, HD // 2, 2)
        x1, x2 = tr[..., 0], tr[..., 1]
        c = cos[None, None]
        sn_ = sin[None, None]
        o1 = x1 * c - x2 * sn_
        o2 = x1 * sn_ + x2 * c
        return np.stack([o1, o2], axis=-1).reshape(b, H, s, HD)

    q1, q2 = q[..., :HD], q[..., HD:]
    k1, k2 = k[..., :HD], k[..., HD:]
    q1, k1 = rope(q1), rope(k1)
    q2, k2 = rope(q2), rope(k2)

    def softm(z):
        z = z - z.max(-1, keepdims=True)
        e = np.exp(z)
        return e / e.sum(-1, keepdims=True)

    m = np.asarray(mask).reshape(1, 1, s, s)
    a1 = softm(np.einsum("bhqd,bhkd->bhqk", q1, k1) * SCALE + m)
    a2 = softm(np.einsum("bhqd,bhkd->bhqk", q2, k2) * SCALE + m)
    a = a1 - float(lbda) * a2
    o = np.einsum("bhqk,bhkd->bhqd", a, v)
    mu = o.mean(-1, keepdims=True)
    var = o.var(-1, keepdims=True)
    o = (o - mu) / np.sqrt(var + 1e-5)
    o = o * ln_gamma[None, :, None, :] + ln_beta[None, :, None, :]
    o = o * OUT_MULT
    o = o.transpose(0, 2, 1, 3).reshape(b, s, d)
    return (o @ W_o).astype(np.float32)


def kernel(x, mask, cos, sin, W_qkv, W_o, ln_gamma, ln_beta, lbda):
    global LAST_RESULTS, LAST_EXEC_NS
    x = np.asarray(x, dtype=np.float32)
    cos = np.asarray(cos, dtype=np.float32)
    sin = np.asarray(sin, dtype=np.float32)
    W_qkv = np.asarray(W_qkv, dtype=np.float32)
    W_o = np.asarray(W_o, dtype=np.float32)
    ln_gamma = np.asarray(ln_gamma, dtype=np.float32)
    ln_beta = np.asarray(ln_beta, dtype=np.float32)
    lbda_f = float(np.asarray(lbda))

    if not _mask_is_causal(mask):
        return _numpy_reference(x, mask, cos, sin, W_qkv, W_o,
                                ln_gamma, ln_beta, lbda_f)

    from concourse.bass_utils import run_bass_kernel_spmd

    nc = get_program(S)
    in_maps = [
        make_core_inputs(x, cos, sin, W_qkv, W_o, ln_gamma, lbda_f, c)
        for c in range(NCORES)
    ]
    kwargs = {"trace": TRACE}
    if TRACE and TRACE_DIR:
        kwargs["tmpdir"] = TRACE_DIR
    res = run_bass_kernel_spmd(nc, in_maps, core_ids=list(range(NCORES)),
                               **kwargs)
    LAST_RESULTS = res
    LAST_EXEC_NS = getattr(res, "exec_time_ns", None)

    outf = np.zeros((B, S, D), dtype=np.float32)
    for c in range(NCORES):
        outf[c // 4] += res.results[c]["out"].astype(np.float32)
    # ln_beta rank-1 term: (beta * OUT_MULT) @ W_o added to every token
    beta_term = (ln_beta.reshape(-1) * OUT_MULT) @ W_o
    outf += beta_term[None, None, :]
    return outf
